# revision 21
# baseline (speedup 1.0000x reference)
"""BlockSparseRingMultiheadDilatedAttention Trainium2 kernel.

Problem (hardcoded): B=1, N=8192, E=1024, H=16 heads, D=64.
Two dilated groups: g0 = heads 0-7, seg 2048, dilation 1;
                    g1 = heads 8-15, seg 4096, dilation 2, offset 1 (odd positions).
Causal within each (gathered) segment.

Sharding over 8 cores (uniform SPMD program, per-core data):
  core c: a = c%2, sc = c//2, b = c%4, rc = c//4
    g0: seg sc (rows 2048*sc .. +2048), heads 4a..4a+4   (4 blocks of [2048 x 2048])
    g1: seg rc odd rows (gathered, 2048 rows), heads 8+2b..+2 (2 blocks)
  Host pre-slices inputs (bf16 cast, odd-row gather, weight head slices) so the
  device program is identical on every core.  Host sums the per-core partial
  output projections (disjoint head contributions) and adds bo.

Device dataflow per core (host pre-transposes X/W so all DMAs are plain
contiguous loads):
  - X^T / W^T tiles (embedding on partitions) loaded with large plain DMAs
  - qT/kT/vT = W^T-stationary matmuls (heads packed in pairs -> K=128, M=128),
    accumulated in paired 2-bank PSUM tiles, one ACT copy+bias per [128,1024]
  - V natural via PE transpose of vT, with an appended ones column
  - S^T tile pairs = K Q^T (per head) into 2-bank PSUM; ONE exp per [128,1024]
    on ACT (scale=1/8, bf16 out); causal masking via DVE mul with 0/1 masks on
    the 2 diagonal pairs per tq chunk; S-pair(i+1) emitted before O-pair(i)
    so the PE has runway while ACT computes the exp
  - O'^T accumulation = [V|1]^T P^T in PSUM; row 64 = softmax denominators
  - normalize via DVE reciprocal + gpsimd partition_broadcast + DVE mul (reads
    PSUM directly) -> O^T (bf16)
  - y = sum_pairs (O^T_pair)^T @ Wo^T_pair into 2-bank PSUM, fp32 out
"""

import numpy as np
import ml_dtypes

BF16 = ml_dtypes.bfloat16

SEG = 2048          # rows per attention block (both groups, post-gather)
E = 1024            # embedding
NQ = 512            # tq chunk (one PSUM bank of fp32)
NTQ = SEG // NQ     # 4 tq chunks per block
NTK = SEG // 128    # 16 tk chunks per block
ECH = E // 128      # 8 embedding chunks

_CACHE = {}


def _build_program():
    import concourse.bacc as bacc
    import concourse.mybir as mybir
    import concourse.tile as tile

    dt = mybir.dt
    nc = bacc.Bacc("TRN2", target_bir_lowering=False, debug=False,
                   enable_asserts=False)

    # ---- DRAM I/O (uniform across cores; host slices per core) ----
    xs = {}
    for sel in ("a", "b"):      # a = g0 rows, b = g1 gathered odd rows
        for inp in ("q", "k", "v"):
            xs[(sel, inp)] = nc.dram_tensor(
                f"x{sel}_{inp}", [E, SEG], dt.bfloat16, kind="ExternalInput").ap()
    ws = {inp: nc.dram_tensor(f"w{inp}", [E, 384], dt.bfloat16,
                              kind="ExternalInput").ap()
          for inp in ("q", "k", "v")}
    wo = nc.dram_tensor("wo", [384, E], dt.bfloat16, kind="ExternalInput").ap()
    bs = {inp: nc.dram_tensor(f"b{inp}", [384, 1], dt.float32,
                              kind="ExternalInput").ap()
          for inp in ("q", "k", "v")}
    y0 = nc.dram_tensor("y0", [SEG, E], dt.float32, kind="ExternalOutput").ap()
    y1 = nc.dram_tensor("y1", [SEG, E], dt.float32, kind="ExternalOutput").ap()

    with tile.TileContext(nc) as tc:
        from contextlib import ExitStack
        with ExitStack() as ctx:
            const = ctx.enter_context(tc.tile_pool(name="const", bufs=1))
            wpool = ctx.enter_context(tc.tile_pool(name="wpool", bufs=1))
            xtp = ctx.enter_context(tc.tile_pool(name="xtp", bufs=2))
            qkt = ctx.enter_context(tc.tile_pool(name="qkt", bufs=1))
            vtp = ctx.enter_context(tc.tile_pool(name="vtp", bufs=2))
            vnat = ctx.enter_context(tc.tile_pool(name="vnat", bufs=1))
            otp = ctx.enter_context(tc.tile_pool(name="otp", bufs=1))
            ptp = ctx.enter_context(tc.tile_pool(name="ptp", bufs=4))
            smallp = ctx.enter_context(tc.tile_pool(name="smallp", bufs=4))
            ypool = ctx.enter_context(tc.tile_pool(name="ypool", bufs=4))
            ps_mm = ctx.enter_context(
                tc.tile_pool(name="ps_mm", bufs=3, space="PSUM"))
            ps_acc = ctx.enter_context(
                tc.tile_pool(name="ps_acc", bufs=2, space="PSUM"))

            # ---- constants: identity (PE transpose), causal tile masks ----
            ident = const.tile([128, 128], dt.bfloat16, tag="ident")
            nc.gpsimd.memset(ident, 1.0)
            nc.gpsimd.affine_select(
                out=ident, in_=ident, compare_op=mybir.AluOpType.is_equal,
                fill=0.0, base=0, pattern=[[-1, 128]], channel_multiplier=1)
            masks2 = []
            for di0 in (0, 2):
                m = const.tile([128, 2 * NQ], dt.bfloat16, tag=f"mask{di0}")
                nc.gpsimd.memset(m, 1.0)
                # half hh covers di = di0 + hh: keep where f - p - 128*di >= 0
                nc.gpsimd.affine_select(
                    out=m.rearrange("p (hh f) -> p hh f", hh=2),
                    in_=m.rearrange("p (hh f) -> p hh f", hh=2),
                    compare_op=mybir.AluOpType.is_ge,
                    fill=0.0, base=-128 * di0, pattern=[[-128, 2], [1, NQ]],
                    channel_multiplier=-1)
                masks2.append(m)

            # ---- weights (host-pre-transposed; plain DMA loads) ----
            # Startup-ordered: first xT(a,q) chunk + Wq land first so the
            # first projection matmuls can begin ~1.3MB into the load stream.
            xt_first = xtp.tile([128, ECH * SEG], dt.bfloat16, tag="xt",
                                name="xt_first")
            nc.sync.dma_start(out=xt_first[:, 0:SEG],
                              in_=xs[("a", "q")][0:128, :])
            # wTa[inp]: [128 e, ECH*384]; chunk ec pair p at 384*ec + 128*p
            wTa = {}

            def load_w(inp):
                t = wpool.tile([128, 384 * ECH], dt.bfloat16,
                               tag=f"wT_{inp}", name=f"wT_{inp}")
                for ec in range(ECH):
                    nc.sync.dma_start(
                        out=t[:, 384 * ec:384 * (ec + 1)],
                        in_=ws[inp][128 * ec:128 * (ec + 1), :])
                wTa[inp] = t

            load_w("q")
            for ec in range(1, ECH):
                nc.sync.dma_start(
                    out=xt_first[:, SEG * ec:SEG * (ec + 1)],
                    in_=xs[("a", "q")][128 * ec:128 * (ec + 1), :])
            load_w("k")
            load_w("v")
            # wT[inp][p]: 3D view [128 e, ECH, 128 d]; chunk ec = [:, ec, :]
            wT = {inp: [wTa[inp].rearrange("p (ec x) -> p ec x", x=384)
                        [:, :, 128 * p:128 * (p + 1)]
                        for p in range(3)] for inp in ("q", "k", "v")}
            # woT[p]: [128 (2 heads d), 1024 j]
            woT = []
            for p in range(3):
                t = wpool.tile([128, E], dt.bfloat16, tag=f"woT_{p}")
                nc.sync.dma_start(out=t, in_=wo[128 * p:128 * (p + 1), :])
                woT.append(t)
            # biases -> SBUF [128,1] per (inp, pair)
            bsb = {}
            for inp in ("q", "k", "v"):
                for p in range(3):
                    t = wpool.tile([128, 1], dt.float32, tag=f"b_{inp}_{p}")
                    nc.sync.dma_start(
                        out=t, in_=bs[inp][128 * p:128 * (p + 1), :])
                    bsb[(inp, p)] = t

            # ---- persistent per-pair activations ----
            qT = [qkt.tile([128, SEG], dt.bfloat16, tag=f"qT{p}", name=f"qT{p}")
                  for p in range(3)]
            kT = [qkt.tile([128, SEG], dt.bfloat16, tag=f"kT{p}", name=f"kT{p}")
                  for p in range(3)]
            # V natural + ones col: per pair [128, 16*130]; chunk i at 130*i,
            # head h lhsT = [:, 130*i + 65*h : +65]
            vn = [vnat.tile([128, NTK * 130], dt.bfloat16, tag=f"vn{p}", name=f"vn{p}")
                  for p in range(3)]
            oT = [otp.tile([128, SEG], dt.bfloat16, tag=f"oT{p}", name=f"oT{p}")
                  for p in range(3)]

            # ---- Phase A: projections ----
            for sel in ("a", "b"):
                pairs = (0, 1) if sel == "a" else (2,)
                for inp in ("q", "k", "v"):
                    if sel == "a" and inp == "q":
                        xt = xt_first
                    else:
                        xt = xtp.tile([128, ECH * SEG], dt.bfloat16, tag="xt")
                        for ec in range(ECH):
                            nc.sync.dma_start(
                                out=xt[:, SEG * ec:SEG * (ec + 1)],
                                in_=xs[(sel, inp)][128 * ec:128 * (ec + 1), :])
                    for p in pairs:
                        if inp == "v":
                            dst = vtp.tile([128, SEG], dt.bfloat16, tag="vT")
                        else:
                            dst = (qT if inp == "q" else kT)[p]
                        for t2 in range(NTQ // 2):
                            acc = ps_mm.tile([128, 2 * NQ], dt.float32,
                                             tag="mm")
                            for half in range(2):
                                for ec in range(ECH):
                                    nc.tensor.matmul(
                                        acc[:, NQ * half:NQ * (half + 1)],
                                        wT[inp][p][:, ec, :],
                                        xt[:, SEG * ec + NQ * (2 * t2 + half):
                                           SEG * ec + NQ * (2 * t2 + half + 1)],
                                        start=(ec == 0), stop=(ec == ECH - 1))
                            nc.scalar.activation(
                                dst[:, 2 * NQ * t2:2 * NQ * (t2 + 1)], acc,
                                mybir.ActivationFunctionType.Identity,
                                bias=bsb[(inp, p)], scale=1.0)
                        if inp == "v":
                            # transpose vT -> V natural (+ ones columns)
                            for i in range(NTK):
                                ptr = ps_acc.tile([128, 128], dt.bfloat16,
                                                  tag="acc", name="ptr")
                                nc.tensor.transpose(
                                    ptr, dst[:, 128 * i:128 * (i + 1)], ident)
                                dsts = vn[p][:, 130 * i:130 * i + 130]
                                dv = dsts.rearrange("p (h x) -> p h x", h=2)
                                nc.vector.tensor_copy(
                                    dv[:, :, 0:64],
                                    ptr.rearrange("p (h d) -> p h d", h=2))
                            ones_view = vn[p].rearrange(
                                "p (k x) -> p k x", x=130)
                            nc.gpsimd.memset(ones_view[:, :, 64:65], 1.0)
                            nc.gpsimd.memset(ones_view[:, :, 129:130], 1.0)

            # ---- Phase B: attention blocks ----
            # Pairs of tk-chunks share one 2-bank PSUM tile and one exp.
            # Emission is software-pipelined: S-pair(i2+1) is emitted before
            # O-pair(i2) so the PE has runway while ACT computes the exp.
            def attention_pair(p):
                for h in range(2):
                    hp = 64 * h
                    for j in (3, 2, 1, 0):
                        n2 = 2 * (j + 1)
                        acc_o = ps_acc.tile([128, NQ], dt.float32, tag="acc")

                        def s_pair(i2):
                            s = ps_mm.tile([128, 2 * NQ], dt.float32,
                                           tag="mm", name="s")
                            for half in range(2):
                                i = 2 * i2 + half
                                nc.tensor.matmul(
                                    s[:, NQ * half:NQ * (half + 1)],
                                    kT[p][hp:hp + 64, 128 * i:128 * (i + 1)],
                                    qT[p][hp:hp + 64, NQ * j:NQ * (j + 1)],
                                    start=True, stop=True)
                            return s

                        def exp_pair(i2, s):
                            pt = ptp.tile([128, 2 * NQ], dt.bfloat16,
                                          tag="pt", name="pt")
                            nc.scalar.activation(
                                pt, s, mybir.ActivationFunctionType.Exp,
                                bias=0.0, scale=0.125)
                            if i2 >= 2 * j:       # diagonal pair
                                nc.vector.tensor_mul(
                                    pt, pt, masks2[i2 - 2 * j])
                            return pt

                        def o_pair(i2, pt):
                            for half in range(2):
                                i = 2 * i2 + half
                                nc.tensor.matmul(
                                    acc_o[0:65, :],
                                    vn[p][:, 130 * i + 65 * h:
                                          130 * i + 65 * h + 65],
                                    pt[:, NQ * half:NQ * (half + 1)],
                                    start=(i == 0), stop=(i == n2 * 2 - 1))

                        prev = None
                        for i2 in range(n2):
                            s = s_pair(i2)
                            if prev is not None:
                                o_pair(prev[0], prev[1])
                            prev = (i2, exp_pair(i2, s))
                        o_pair(prev[0], prev[1])

                        rj = smallp.tile([1, NQ], dt.float32, tag="rj",
                                         name="rj")
                        nc.vector.reciprocal(rj, acc_o[64:65, :])
                        rb = smallp.tile([64, NQ], dt.float32, tag="rb",
                                         name="rb")
                        nc.gpsimd.partition_broadcast(rb, rj)
                        nc.vector.tensor_mul(
                            oT[p][hp:hp + 64, NQ * j:NQ * (j + 1)],
                            acc_o[0:64, :], rb)

            # ---- Phase C: output projection ----
            def out_proj(ydram, pairs):
                for m in range(NTK):
                    accy = ps_mm.tile([128, 2 * NQ], dt.float32, tag="mm",
                                      name="accy")
                    for jc in range(2):
                        for idx, p in enumerate(pairs):
                            nc.tensor.matmul(
                                accy[:, NQ * jc:NQ * (jc + 1)],
                                oT[p][:, 128 * m:128 * (m + 1)],
                                woT[p][:, NQ * jc:NQ * (jc + 1)],
                                start=(idx == 0), stop=(idx == len(pairs) - 1))
                    ysb = ypool.tile([128, 2 * NQ], dt.float32, tag="ysb")
                    nc.vector.tensor_copy(ysb, accy)
                    nc.sync.dma_start(
                        out=ydram[128 * m:128 * (m + 1), :], in_=ysb)

            attention_pair(0)
            attention_pair(1)
            attention_pair(2)
            out_proj(y0, (0, 1))
            out_proj(y1, (2,))

    nc.compile()
    return nc


def _get_program():
    if "nc" not in _CACHE:
        _CACHE["nc"] = _build_program()
    return _CACHE["nc"]


def _prep_inputs(query, key, value, Wq, bq, Wk, bk, Wv, bv, Wo, bo):
    """Build the 8 per-core input maps (host-side slicing + bf16 cast)."""
    q = np.asarray(query, np.float32).reshape(8192, 1024).astype(BF16)
    k = np.asarray(key, np.float32).reshape(8192, 1024).astype(BF16)
    v = np.asarray(value, np.float32).reshape(8192, 1024).astype(BF16)
    wq = np.asarray(Wq, np.float32).astype(BF16)
    wk = np.asarray(Wk, np.float32).astype(BF16)
    wv = np.asarray(Wv, np.float32).astype(BF16)
    wo_f = np.asarray(Wo, np.float32).astype(BF16)
    bqf = np.asarray(bq, np.float32)
    bkf = np.asarray(bk, np.float32)
    bvf = np.asarray(bv, np.float32)

    qT, kT, vT = q.T, k.T, v.T  # [1024, 8192] views
    in_maps = []
    for c in range(8):
        a, sc, b, rc = c % 2, c // 2, c % 4, c // 4
        rows_g0 = slice(2048 * sc, 2048 * (sc + 1))
        rows_g1 = slice(4096 * rc + 1, 4096 * (rc + 1), 2)
        hrows = np.r_[256 * a:256 * a + 256, 512 + 128 * b:512 + 128 * b + 128]
        m = {
            "xa_q": np.ascontiguousarray(qT[:, rows_g0]),
            "xa_k": np.ascontiguousarray(kT[:, rows_g0]),
            "xa_v": np.ascontiguousarray(vT[:, rows_g0]),
            "xb_q": np.ascontiguousarray(qT[:, rows_g1]),
            "xb_k": np.ascontiguousarray(kT[:, rows_g1]),
            "xb_v": np.ascontiguousarray(vT[:, rows_g1]),
            "wq": np.ascontiguousarray(wq[hrows].T),
            "wk": np.ascontiguousarray(wk[hrows].T),
            "wv": np.ascontiguousarray(wv[hrows].T),
            "wo": np.ascontiguousarray(wo_f[:, hrows].T),
            "bq": np.ascontiguousarray(bqf[hrows]).reshape(384, 1),
            "bk": np.ascontiguousarray(bkf[hrows]).reshape(384, 1),
            "bv": np.ascontiguousarray(bvf[hrows]).reshape(384, 1),
        }
        in_maps.append(m)
    return in_maps


def _combine(results, bo):
    y = np.zeros((8192, 1024), np.float32)
    for c in range(8):
        sc, rc = c // 2, c // 4
        y[2048 * sc:2048 * (sc + 1)] += results[c]["y0"]
        y[4096 * rc + 1:4096 * (rc + 1):2] += results[c]["y1"]
    y += np.asarray(bo, np.float32)
    return y.reshape(1, 8192, 1024)


def kernel(query, key, value, Wq, bq, Wk, bk, Wv, bv, Wo, bo,
           _trace=False, _trace_cores=None):
    from concourse import bass_utils
    nc = _get_program()
    in_maps = _prep_inputs(query, key, value, Wq, bq, Wk, bk, Wv, bv, Wo, bo)
    res = bass_utils.run_bass_kernel_spmd(
        nc, in_maps, core_ids=list(range(8)),
        trace=_trace, trace_cores=_trace_cores)
    _CACHE["last_results"] = res
    return _combine(res.results, bo)
